# revision 41
# baseline (speedup 1.0000x reference)
"""Trainium2 Bass kernel: batched scaled-dot-product attention returning
(ctx, probs), data-parallel over 8 NeuronCores (2 batches/core).

Per-core dataflow (B=2 local batches, L=2048, D=128):
  - load q,k,v; PE-transpose q,k -> qT,kT [128d, L] in SBUF (float32r)
  - probs path, per 128-row q-tile: scores = qT_i^T @ kT (fp32r matmuls into
    PSUM), ScalarE exp(scale*x) with accum_out giving row sums, VectorE
    normalize by reciprocal, DMA out.
  - ctx path, per 512-col q-block: build expT [k, q] either by
    (V2) PE-transposing the exp tiles, evacuating PSUM->SBUF on ScalarE/
    VectorE alternately, or (V1) recomputing scoresT = kT_j^T @ qT on the PE
    and running a second exp pass; then ctxT += v_j^T . expT_j accumulated in
    one PSUM bank; PE-transpose back to [q, d], VectorE scale by the
    reciprocal, DMA out.
"""

import numpy as np

import concourse.bass as bass
import concourse.tile as tile
from concourse import bacc, masks, mybir
from concourse.bass_utils import run_bass_kernel_spmd

B, L, D = 16, 2048, 128
NCORES = 8
BPC = B // NCORES          # batches per core
NT = L // 128              # 128-row tiles per batch (16)
QB = 512                   # q-block width for the ctx path
NQB = L // QB              # q-blocks per batch (4)
SCALE = float(np.float32(1.0) / np.sqrt(np.float32(D)))

f32 = mybir.dt.float32
f32r = mybir.dt.float32r
EXP = mybir.ActivationFunctionType.Exp
COPY = mybir.ActivationFunctionType.Copy

V2 = True       # ebuf dtype f32r + transpose-route enabled
# ctx-path expT source per k-tile j: j in TRANS_JS -> PE-transpose of the exp1
# output (evacuated on VectorE); else -> scoresT matmul + second exp on ScalarE.
TRANS_JS = frozenset(range(8, 16))

_cache = {}


def _emit_stage_a(nc, pools, dram, b):
    (const_pool, qkv_pool, vr_pool, qt_pool, expt_pool, e_pool, p_pool,
     cts_pool, ctx_pool, sc_pool, sps_pool, stps_pool, acc_pool) = pools
    q_d, k_d, v_d, probs_d, ctx_d = dram
    ident = const_pool.ident

    # ---- load q,k,v; first-needed chunks first so compute starts early ----
    qsb = qkv_pool.tile([128, NT, 128], f32, tag="qsb")
    ksb = qkv_pool.tile([128, NT, 128], f32, tag="ksb")
    vsb = qkv_pool.tile([128, NT, 128], f32, tag="vsb")
    q_ap = q_d[b].rearrange("(t p) d -> p t d", p=128)
    k_ap = k_d[b].rearrange("(t p) d -> p t d", p=128)
    v_ap = v_d[b].rearrange("(t p) d -> p t d", p=128)
    loads = [(k_ap, ksb, 0), (q_ap, qsb, 0), (k_ap, ksb, 1), (k_ap, ksb, 2),
             (k_ap, ksb, 3)]
    loads += [(q_ap, qsb, c) for c in range(1, 4)]
    loads += [(v_ap, vsb, c) for c in range(4)]
    for ap, dst, c in loads:
        nc.sync.dma_start(
            dst[:, c * 4 : (c + 1) * 4, :], ap[:, c * 4 : (c + 1) * 4, :]
        )

    # ---- PE-transpose q,k into [128 d, L] f32r ----
    qTr = qt_pool.tile([128, L], f32r, tag="qTr")
    kTr = qt_pool.tile([128, L], f32r, tag="kTr")
    tgroups = [(ksb, kTr, 0), (qsb, qTr, 0), (ksb, kTr, 1), (ksb, kTr, 2),
               (ksb, kTr, 3)]
    tgroups += [(qsb, qTr, g) for g in range(1, 4)]
    for sb_t, dst, g in tgroups:
        tp = acc_pool.tile([128, 512], f32, tag="acc")
        for u in range(4):
            t = g * 4 + u
            nc.tensor.transpose(
                tp[:, u * 128 : (u + 1) * 128], sb_t[:, t, :], ident[:]
            )
        nc.vector.tensor_copy(dst[:, g * 512 : (g + 1) * 512], tp[:])

    # cast v after the q/k transpose evacs: v loads land last, and an early
    # cast would head-of-line block the DVE queue behind the v DMA
    v_r = vr_pool.tile([128, NT, 128], f32r)
    nc.vector.tensor_copy(v_r[:], vsb[:])
    return qTr, kTr, v_r


def _ctx_out(nc, pools, ctx_d, b, qb, iis, ctp, rrows):
    """Scale the transposed ctx tiles for q-tiles `iis` (contiguous) and
    store them with one merged DMA (interleaved-row 3D access pattern)."""
    ctx_pool = pools[8]
    nt = len(iis)
    ct = ctx_pool.tile([128, nt, 128], f32, tag="ct")
    for x, ii in enumerate(iis):
        nc.vector.tensor_scalar_mul(
            ct[:, x, :], ctp[:, x * 128 : (x + 1) * 128], rrows[ii][:]
        )
    i0 = qb * 4 + iis[0]
    dst = ctx_d[b, i0 * 128 : (i0 + nt) * 128, :].rearrange(
        "(c p) d -> p c d", p=128
    )
    nc.sync.dma_start(dst, ct[:])


def _emit_blocks(nc, pools, dram, b, stage, last):
    (const_pool, qkv_pool, vr_pool, qt_pool, expt_pool, e_pool, p_pool,
     cts_pool, ctx_pool, sc_pool, sps_pool, stps_pool, acc_pool) = pools
    q_d, k_d, v_d, probs_d, ctx_d = dram
    qTr, kTr, v_r = stage
    ident = const_pool.ident
    trans_js = sorted(TRANS_JS)
    mm_js = sorted(set(range(NT)) - TRANS_JS)

    def sT_exp2(qb):
        """scoresT matmul + second exp for the matmul-route k-tiles; returns
        this block's expT tile and ctx accumulator halves."""
        expT = expt_pool.tile([128, NT * QB], f32r)
        if last and qb == NQB - 1:
            # split ctx accumulator: half A (q 256:512) closes early so its
            # output chain overlaps the remaining exp work
            cTa = acc_pool.tile([128, QB // 2], f32, tag="acc")
            cTb = acc_pool.tile([128, QB // 2], f32, tag="acc")
            cT = (cTa, cTb)
        else:
            cTf = acc_pool.tile([128, QB], f32, tag="acc")
            cT = (cTf,)
        for j in mm_js:
            sT = stps_pool.tile([128, QB], f32, tag="stp")
            nc.tensor.matmul(
                sT[:],
                kTr[:, j * 128 : (j + 1) * 128],
                qTr[:, qb * QB : (qb + 1) * QB],
                start=True,
                stop=True,
            )
            nc.scalar.activation(
                expT[:, j * QB : (j + 1) * QB], sT[:], EXP, scale=SCALE
            )
        return expT, cT

    def fold(cTt, expT, js, col0, colw, start, stop):
        for jx, j in enumerate(js):
            nc.tensor.matmul(
                cTt[:, 0:colw],
                v_r[:, j, :],
                expT[:, j * QB + col0 : j * QB + col0 + colw],
                start=(start and jx == 0),
                stop=(stop and jx == len(js) - 1),
            )

    def ctx_chain(cTt, expT, qb, col0, colw, iis, rrows):
        """trans-folds (closing the group) + evacuate + transpose + scale/DMA
        for q columns [col0, col0+colw) of the block."""
        fold(cTt, expT, trans_js, col0, colw, start=False, stop=True)
        cT_sb = cts_pool.tile([128, QB], f32)
        nc.vector.tensor_copy(cT_sb[:, 0:colw], cTt[:, 0:colw])
        ctp = stps_pool.tile([128, QB], f32, tag="stp")
        for x in range(colw // 128):
            nc.tensor.transpose(
                ctp[:, x * 128 : (x + 1) * 128],
                cT_sb[:, x * 128 : (x + 1) * 128],
                ident[:],
            )
        _ctx_out(nc, pools, ctx_d, b, qb, iis, ctp, rrows)

    state = sT_exp2(0)
    for qb in range(NQB):
        last_blk = last and qb == NQB - 1
        expT, cT = state
        expT_v = expT[:].rearrange("p (j q) -> p j q", q=QB)
        rrows = {}

        # ---- probs path: one 128-row q-tile at a time ----
        # scores are computed in two [128, 1024] half-tiles (2 PSUM banks
        # each, double-buffered) so the PE can fill half B / next tile's
        # half A while ScalarE is still reading the previous half.
        ii_order = [3, 2, 1, 0] if last_blk else [0, 1, 2, 3]
        for pos, ii in enumerate(ii_order):
            i = qb * 4 + ii
            ebuf = e_pool.tile([128, L], f32r)
            # the very first tile uses 512-wide exp chunks so the first
            # activation starts as soon as one k-transpose group is ready
            fine = b == 0 and qb == 0 and ii == 0
            zparts = []
            for h in range(2):
                s_ps = sps_pool.tile([128, L // 2], f32, tag="sps")
                for n in range(2):
                    nc.tensor.matmul(
                        s_ps[:, n * 512 : (n + 1) * 512],
                        qTr[:, i * 128 : (i + 1) * 128],
                        kTr[:, (2 * h + n) * 512 : (2 * h + n + 1) * 512],
                        start=True,
                        stop=True,
                    )
                    if fine:
                        zh = sc_pool.tile([128, 1], f32, tag="zrow")
                        nc.scalar.activation(
                            ebuf[:, (2 * h + n) * 512 : (2 * h + n + 1) * 512],
                            s_ps[:, n * 512 : (n + 1) * 512],
                            EXP,
                            scale=SCALE,
                            accum_out=zh[:],
                        )
                        zparts.append(zh)
                if not fine:
                    zh = sc_pool.tile([128, 1], f32, tag="zrow")
                    nc.scalar.activation(
                        ebuf[:, h * (L // 2) : (h + 1) * (L // 2)],
                        s_ps[:],
                        EXP,
                        scale=SCALE,
                        accum_out=zh[:],
                    )
                    zparts.append(zh)
            while len(zparts) > 1:
                zn = sc_pool.tile([128, 1], f32, tag="zsum")
                nc.vector.tensor_add(zn[:], zparts[0][:], zparts[1][:])
                zparts = zparts[2:] + [zn]
            rrow = sc_pool.tile([128, 1], f32, tag="rrow")
            nc.vector.reciprocal(rrow[:], zparts[0][:])
            rrows[ii] = rrow

            def norm_dma(nch):
                pt = p_pool.tile([128, L], f32, tag="pt")
                cw = L // nch
                for ch in range(nch):
                    nc.vector.tensor_scalar_mul(
                        pt[:, ch * cw : (ch + 1) * cw],
                        ebuf[:, ch * cw : (ch + 1) * cw],
                        rrow[:],
                    )
                    nc.sync.dma_start(
                        probs_d[b, i * 128 : (i + 1) * 128,
                                ch * cw : (ch + 1) * cw],
                        pt[:, ch * cw : (ch + 1) * cw],
                    )

            def trans_evac():
                for g in range(len(trans_js) // 4):
                    js = trans_js[g * 4 : g * 4 + 4]
                    tp = stps_pool.tile([128, 512], f32r, tag="stp")
                    for u, j in enumerate(js):
                        nc.tensor.transpose(
                            tp[:, u * 128 : (u + 1) * 128],
                            ebuf[:, j * 128 : (j + 1) * 128],
                            const_pool.ident_r[:],
                        )
                    dst = expT_v[:, js[0] : js[0] + 4, ii * 128 : (ii + 1) * 128]
                    src = tp[:].rearrange("p (u x) -> p u x", x=128)
                    nc.vector.tensor_copy(dst, src)

            if last_blk and pos == 3:
                # final tile: evacuate first (it gates the last ctx fold),
                # then chunked normalize+DMA, then the half-A output chain
                trans_evac()
                norm_dma(2)
                ctx_chain(cT[0], expT, qb, 256, 256, [2, 3], rrows)
            else:
                norm_dma(1)
                if pos == 1:
                    # fold the matmul-route k-tiles into the ctx accumulator
                    # now: their exp2 outputs are long done, no PE HOL wait
                    if last_blk:
                        fold(cT[0], expT, mm_js, 256, 256, start=True,
                             stop=False)
                        fold(cT[1], expT, mm_js, 0, 256, start=True,
                             stop=False)
                    else:
                        fold(cT[0], expT, mm_js, 0, QB, start=True, stop=False)
                trans_evac()

        # next block's scoresT/exp2 run before this block's trailing folds so
        # the PE/ACT never idle at the block boundary
        if qb + 1 < NQB:
            state = sT_exp2(qb + 1)

        if last_blk:
            ctx_chain(cT[1], expT, qb, 0, 256, [0, 1], rrows)
        else:
            ctx_chain(cT[0], expT, qb, 0, QB, [0, 1, 2, 3], rrows)


def _build(reps=1):
    if reps in _cache:
        return _cache[reps]

    nc = bacc.Bacc("TRN2", target_bir_lowering=False, debug=False)
    q_d = nc.dram_tensor("q", [BPC, L, D], f32, kind="ExternalInput").ap()
    k_d = nc.dram_tensor("k", [BPC, L, D], f32, kind="ExternalInput").ap()
    v_d = nc.dram_tensor("v", [BPC, L, D], f32, kind="ExternalInput").ap()
    probs_d = nc.dram_tensor("probs", [BPC, L, L], f32, kind="ExternalOutput").ap()
    ctx_d = nc.dram_tensor("ctx", [BPC, L, D], f32, kind="ExternalOutput").ap()
    dram = (q_d, k_d, v_d, probs_d, ctx_d)

    with tile.TileContext(nc) as tc:
        with (
            tc.tile_pool(name="const", bufs=1) as const_pool,
            tc.tile_pool(name="qkv", bufs=1) as qkv_pool,
            tc.tile_pool(name="vr", bufs=2) as vr_pool,
            tc.tile_pool(name="qt", bufs=2) as qt_pool,
            tc.tile_pool(name="expt", bufs=2) as expt_pool,
            tc.tile_pool(name="ebuf", bufs=3) as e_pool,
            tc.tile_pool(name="probs", bufs=2) as p_pool,
            tc.tile_pool(name="cts", bufs=2) as cts_pool,
            tc.tile_pool(name="ctxo", bufs=8) as ctx_pool,
            tc.tile_pool(name="scal", bufs=24) as sc_pool,
            tc.tile_pool(name="sps", bufs=2, space="PSUM") as sps_pool,
            tc.tile_pool(name="stps", bufs=2, space="PSUM") as stps_pool,
            tc.tile_pool(name="acc", bufs=2, space="PSUM") as acc_pool,
        ):
            ident = const_pool.tile([128, 128], f32)
            masks.make_identity(nc, ident[:])
            const_pool.ident = ident
            ident_r = const_pool.tile([128, 128], f32r)
            nc.vector.tensor_copy(ident_r[:], ident[:])
            const_pool.ident_r = ident_r
            # warm the Exp activation table off the critical path
            warm = const_pool.tile([128, 1], f32)
            nc.vector.memset(warm[:], 0.0)
            warm2 = const_pool.tile([128, 1], f32)
            nc.scalar.activation(warm2[:], warm[:], EXP)
            # warm the PE clock (HAM) during the first DMA window with dummy
            # matmuls on a zeroed tile so stage-A transposes run at full rate
            wmm = const_pool.tile([128, 128], mybir.dt.bfloat16)
            nc.vector.memset(wmm[:], 0.0)
            wps = acc_pool.tile([128, 512], f32, tag="acc")
            for wi in range(10):
                nc.tensor.matmul(
                    wps[:, 0:128], wmm[:], wmm[:], start=True, stop=True
                )
            pools = (const_pool, qkv_pool, vr_pool, qt_pool, expt_pool, e_pool,
                     p_pool, cts_pool, ctx_pool, sc_pool, sps_pool, stps_pool,
                     acc_pool)
            for r in range(reps):
                stages = [_emit_stage_a(nc, pools, dram, b) for b in range(BPC)]
                for b in range(BPC):
                    _emit_blocks(nc, pools, dram, b, stages[b],
                                 last=(r == reps - 1 and b == BPC - 1))

    nc.compile()
    _cache[reps] = nc
    return nc


def run(q, k, v, reps=1, **spmd_kwargs):
    nc = _build(reps)
    q = np.ascontiguousarray(np.asarray(q, dtype=np.float32))
    k = np.ascontiguousarray(np.asarray(k, dtype=np.float32))
    v = np.ascontiguousarray(np.asarray(v, dtype=np.float32))
    in_maps = [
        {
            "q": q[c * BPC : (c + 1) * BPC],
            "k": k[c * BPC : (c + 1) * BPC],
            "v": v[c * BPC : (c + 1) * BPC],
        }
        for c in range(NCORES)
    ]
    out = run_bass_kernel_spmd(nc, in_maps, core_ids=list(range(NCORES)), **spmd_kwargs)
    ctx = np.concatenate([r["ctx"] for r in out.results], axis=0)
    probs = np.concatenate([r["probs"] for r in out.results], axis=0)
    return (ctx, probs), out


def kernel(q, k, v):
    (ctx, probs), _ = run(q, k, v)
    return (ctx, probs)


# revision 51
# speedup vs baseline: 1.0153x; 1.0153x over previous
"""Trainium2 Bass kernel: batched scaled-dot-product attention returning
(ctx, probs), data-parallel over 8 NeuronCores (2 batches/core).

Per-core dataflow (B=2 local batches, L=2048, D=128):
  - load q,k,v; PE-transpose q,k -> qT,kT [128d, L] in SBUF (float32r)
  - probs path, per 128-row q-tile: scores = qT_i^T @ kT (fp32r matmuls into
    PSUM), ScalarE exp(scale*x) with accum_out giving row sums, VectorE
    normalize by reciprocal, DMA out.
  - ctx path, per 512-col q-block: build expT [k, q] either by
    (V2) PE-transposing the exp tiles, evacuating PSUM->SBUF on ScalarE/
    VectorE alternately, or (V1) recomputing scoresT = kT_j^T @ qT on the PE
    and running a second exp pass; then ctxT += v_j^T . expT_j accumulated in
    one PSUM bank; PE-transpose back to [q, d], VectorE scale by the
    reciprocal, DMA out.
"""

import numpy as np

import concourse.bass as bass
import concourse.tile as tile
from concourse import bacc, masks, mybir
from concourse.bass_utils import run_bass_kernel_spmd

B, L, D = 16, 2048, 128
NCORES = 8
BPC = B // NCORES          # batches per core
NT = L // 128              # 128-row tiles per batch (16)
QB = 512                   # q-block width for the ctx path
NQB = L // QB              # q-blocks per batch (4)
SCALE = float(np.float32(1.0) / np.sqrt(np.float32(D)))

f32 = mybir.dt.float32
f32r = mybir.dt.float32r
EXP = mybir.ActivationFunctionType.Exp

# ctx-path expT source per k-tile j: j in TRANS_JS -> PE-transpose of the exp1
# output (evacuated on VectorE); else -> scoresT matmul + second exp on ScalarE.
# The 8/8 split balances ScalarE and VectorE; measured optimum.
TRANS_JS = frozenset(range(8, 16))

_cache = {}


def _emit_stage_a(nc, pools, dram, b):
    (const_pool, qkv_pool, vr_pool, qt_pool, expt_pool, e_pool, p_pool,
     cts_pool, ctx_pool, sc_pool, sps_pool, stps_pool, acc_pool) = pools
    q_d, k_d, v_d, probs_d, ctx_d = dram
    ident = const_pool.ident

    # ---- load q,k,v; first-needed chunks first so compute starts early ----
    qsb = qkv_pool.tile([128, NT, 128], f32, tag="qsb")
    ksb = qkv_pool.tile([128, NT, 128], f32, tag="ksb")
    vsb = qkv_pool.tile([128, NT, 128], f32, tag="vsb")
    q_ap = q_d[b].rearrange("(t p) d -> p t d", p=128)
    k_ap = k_d[b].rearrange("(t p) d -> p t d", p=128)
    v_ap = v_d[b].rearrange("(t p) d -> p t d", p=128)
    loads = [(k_ap, ksb, 0), (q_ap, qsb, 0), (k_ap, ksb, 1), (k_ap, ksb, 2),
             (k_ap, ksb, 3)]
    loads += [(q_ap, qsb, c) for c in range(1, 4)]
    loads += [(v_ap, vsb, c) for c in range(4)]
    for ap, dst, c in loads:
        nc.sync.dma_start(
            dst[:, c * 4 : (c + 1) * 4, :], ap[:, c * 4 : (c + 1) * 4, :]
        )

    # ---- PE-transpose q,k into [128 d, L] f32r ----
    qTr = qt_pool.tile([128, L], f32r, tag="qTr")
    kTr = qt_pool.tile([128, L], f32r, tag="kTr")
    tgroups = [(ksb, kTr, 0), (qsb, qTr, 0), (ksb, kTr, 1), (ksb, kTr, 2),
               (ksb, kTr, 3)]
    tgroups += [(qsb, qTr, g) for g in range(1, 4)]
    for sb_t, dst, g in tgroups:
        tp = acc_pool.tile([128, 512], f32, tag="acc")
        for u in range(4):
            t = g * 4 + u
            nc.tensor.transpose(
                tp[:, u * 128 : (u + 1) * 128], sb_t[:, t, :], ident[:]
            )
        nc.vector.tensor_copy(dst[:, g * 512 : (g + 1) * 512], tp[:])

    # cast v after the q/k transpose evacs: v loads land last, and an early
    # cast would head-of-line block the DVE queue behind the v DMA
    v_r = vr_pool.tile([128, NT, 128], f32r)
    nc.vector.tensor_copy(v_r[:], vsb[:])
    return qTr, kTr, v_r


def _ctx_out(nc, pools, ctx_d, b, qb, iis, ctp, rrows):
    """Scale the transposed ctx tiles for q-tiles `iis` (contiguous) and
    store them with one merged DMA (interleaved-row 3D access pattern)."""
    ctx_pool = pools[8]
    nt = len(iis)
    ct = ctx_pool.tile([128, nt, 128], f32, tag="ct")
    for x, ii in enumerate(iis):
        nc.vector.tensor_scalar_mul(
            ct[:, x, :], ctp[:, x * 128 : (x + 1) * 128], rrows[ii][:]
        )
    i0 = qb * 4 + iis[0]
    dst = ctx_d[b, i0 * 128 : (i0 + nt) * 128, :].rearrange(
        "(c p) d -> p c d", p=128
    )
    nc.sync.dma_start(dst, ct[:])


def _emit_blocks(nc, pools, dram, b, stage, last):
    (const_pool, qkv_pool, vr_pool, qt_pool, expt_pool, e_pool, p_pool,
     cts_pool, ctx_pool, sc_pool, sps_pool, stps_pool, acc_pool) = pools
    q_d, k_d, v_d, probs_d, ctx_d = dram
    qTr, kTr, v_r = stage
    ident = const_pool.ident
    trans_js = sorted(TRANS_JS)
    mm_js = sorted(set(range(NT)) - TRANS_JS)

    def sT_exp2(qb):
        """scoresT matmul + second exp for the matmul-route k-tiles; returns
        this block's expT tile and ctx accumulator halves."""
        expT = expt_pool.tile([128, NT * QB], f32r)
        if last and qb == NQB - 1:
            # split ctx accumulator: half A (q 256:512) closes early so its
            # output chain overlaps the remaining exp work
            cTa = acc_pool.tile([128, QB // 2], f32, tag="acc")
            cTb = acc_pool.tile([128, QB // 2], f32, tag="acc")
            cT = (cTa, cTb)
        else:
            cTf = acc_pool.tile([128, QB], f32, tag="acc")
            cT = (cTf,)
        for j in mm_js:
            sT = stps_pool.tile([128, QB], f32, tag="stp")
            nc.tensor.matmul(
                sT[:],
                kTr[:, j * 128 : (j + 1) * 128],
                qTr[:, qb * QB : (qb + 1) * QB],
                start=True,
                stop=True,
            )
            nc.scalar.activation(
                expT[:, j * QB : (j + 1) * QB], sT[:], EXP, scale=SCALE
            )
        return expT, cT

    def fold(cTt, expT, js, col0, colw, start, stop):
        for jx, j in enumerate(js):
            nc.tensor.matmul(
                cTt[:, 0:colw],
                v_r[:, j, :],
                expT[:, j * QB + col0 : j * QB + col0 + colw],
                start=(start and jx == 0),
                stop=(stop and jx == len(js) - 1),
            )

    def ctx_chain(cTt, expT, qb, col0, colw, iis, rrows):
        """trans-folds (closing the group) + evacuate + transpose + scale/DMA
        for q columns [col0, col0+colw) of the block."""
        fold(cTt, expT, trans_js, col0, colw, start=False, stop=True)
        cT_sb = cts_pool.tile([128, QB], f32)
        nc.vector.tensor_copy(cT_sb[:, 0:colw], cTt[:, 0:colw])
        ctp = stps_pool.tile([128, QB], f32, tag="stp")
        for x in range(colw // 128):
            nc.tensor.transpose(
                ctp[:, x * 128 : (x + 1) * 128],
                cT_sb[:, x * 128 : (x + 1) * 128],
                ident[:],
            )
        _ctx_out(nc, pools, ctx_d, b, qb, iis, ctp, rrows)

    state = sT_exp2(0)
    for qb in range(NQB):
        last_blk = last and qb == NQB - 1
        expT, cT = state
        expT_v = expT[:].rearrange("p (j q) -> p j q", q=QB)
        rrows = {}

        # ---- probs path: one 128-row q-tile at a time ----
        # scores are computed in two [128, 1024] half-tiles (2 PSUM banks
        # each, double-buffered) so the PE can fill half B / next tile's
        # half A while ScalarE is still reading the previous half.
        ii_order = [3, 2, 1, 0] if last_blk else [0, 1, 2, 3]
        for pos, ii in enumerate(ii_order):
            i = qb * 4 + ii
            ebuf = e_pool.tile([128, L], f32r)
            # the very first tile uses 512-wide exp chunks so the first
            # activation starts as soon as one k-transpose group is ready
            fine = b == 0 and qb == 0 and ii == 0
            zparts = []
            for h in range(2):
                s_ps = sps_pool.tile([128, L // 2], f32, tag="sps")
                for n in range(2):
                    nc.tensor.matmul(
                        s_ps[:, n * 512 : (n + 1) * 512],
                        qTr[:, i * 128 : (i + 1) * 128],
                        kTr[:, (2 * h + n) * 512 : (2 * h + n + 1) * 512],
                        start=True,
                        stop=True,
                    )
                    if fine:
                        zh = sc_pool.tile([128, 1], f32, tag="zrow")
                        nc.scalar.activation(
                            ebuf[:, (2 * h + n) * 512 : (2 * h + n + 1) * 512],
                            s_ps[:, n * 512 : (n + 1) * 512],
                            EXP,
                            scale=SCALE,
                            accum_out=zh[:],
                        )
                        zparts.append(zh)
                if not fine:
                    zh = sc_pool.tile([128, 1], f32, tag="zrow")
                    nc.scalar.activation(
                        ebuf[:, h * (L // 2) : (h + 1) * (L // 2)],
                        s_ps[:],
                        EXP,
                        scale=SCALE,
                        accum_out=zh[:],
                    )
                    zparts.append(zh)
            while len(zparts) > 1:
                zn = sc_pool.tile([128, 1], f32, tag="zsum")
                nc.vector.tensor_add(zn[:], zparts[0][:], zparts[1][:])
                zparts = zparts[2:] + [zn]
            rrow = sc_pool.tile([128, 1], f32, tag="rrow")
            nc.vector.reciprocal(rrow[:], zparts[0][:])
            rrows[ii] = rrow

            def norm_dma(nch):
                pt = p_pool.tile([128, L], f32, tag="pt")
                cw = L // nch
                for ch in range(nch):
                    nc.vector.tensor_scalar_mul(
                        pt[:, ch * cw : (ch + 1) * cw],
                        ebuf[:, ch * cw : (ch + 1) * cw],
                        rrow[:],
                    )
                    nc.sync.dma_start(
                        probs_d[b, i * 128 : (i + 1) * 128,
                                ch * cw : (ch + 1) * cw],
                        pt[:, ch * cw : (ch + 1) * cw],
                    )

            def trans_evac():
                for g in range(len(trans_js) // 4):
                    js = trans_js[g * 4 : g * 4 + 4]
                    tp = stps_pool.tile([128, 512], f32r, tag="stp")
                    for u, j in enumerate(js):
                        nc.tensor.transpose(
                            tp[:, u * 128 : (u + 1) * 128],
                            ebuf[:, j * 128 : (j + 1) * 128],
                            const_pool.ident_r[:],
                        )
                    dst = expT_v[:, js[0] : js[0] + 4, ii * 128 : (ii + 1) * 128]
                    src = tp[:].rearrange("p (u x) -> p u x", x=128)
                    nc.vector.tensor_copy(dst, src)

            if last_blk and pos == 3:
                # final tile: evacuate first (it gates the last ctx fold),
                # then chunked normalize+DMA, then the half-A output chain
                trans_evac()
                norm_dma(2)
                ctx_chain(cT[0], expT, qb, 256, 256, [2, 3], rrows)
            else:
                norm_dma(1)
                if pos == 1:
                    # fold the matmul-route k-tiles into the ctx accumulator
                    # now: their exp2 outputs are long done, no PE HOL wait
                    if last_blk:
                        fold(cT[0], expT, mm_js, 256, 256, start=True,
                             stop=False)
                        fold(cT[1], expT, mm_js, 0, 256, start=True,
                             stop=False)
                    else:
                        fold(cT[0], expT, mm_js, 0, QB, start=True, stop=False)
                trans_evac()

        # next block's scoresT/exp2 run before this block's trailing folds so
        # the PE/ACT never idle at the block boundary
        if qb + 1 < NQB:
            state = sT_exp2(qb + 1)

        if last_blk:
            ctx_chain(cT[1], expT, qb, 0, 256, [0, 1], rrows)
        else:
            ctx_chain(cT[0], expT, qb, 0, QB, [0, 1, 2, 3], rrows)


def _build(reps=1):
    if reps in _cache:
        return _cache[reps]

    nc = bacc.Bacc("TRN2", target_bir_lowering=False, debug=False)
    q_d = nc.dram_tensor("q", [BPC, L, D], f32, kind="ExternalInput").ap()
    k_d = nc.dram_tensor("k", [BPC, L, D], f32, kind="ExternalInput").ap()
    v_d = nc.dram_tensor("v", [BPC, L, D], f32, kind="ExternalInput").ap()
    probs_d = nc.dram_tensor("probs", [BPC, L, L], f32, kind="ExternalOutput").ap()
    ctx_d = nc.dram_tensor("ctx", [BPC, L, D], f32, kind="ExternalOutput").ap()
    dram = (q_d, k_d, v_d, probs_d, ctx_d)

    with tile.TileContext(nc) as tc:
        with (
            tc.tile_pool(name="const", bufs=1) as const_pool,
            tc.tile_pool(name="qkv", bufs=1) as qkv_pool,
            tc.tile_pool(name="vr", bufs=2) as vr_pool,
            tc.tile_pool(name="qt", bufs=2) as qt_pool,
            tc.tile_pool(name="expt", bufs=2) as expt_pool,
            tc.tile_pool(name="ebuf", bufs=3) as e_pool,
            tc.tile_pool(name="probs", bufs=4) as p_pool,
            tc.tile_pool(name="cts", bufs=2) as cts_pool,
            tc.tile_pool(name="ctxo", bufs=2) as ctx_pool,
            tc.tile_pool(name="scal", bufs=24) as sc_pool,
            tc.tile_pool(name="sps", bufs=2, space="PSUM") as sps_pool,
            tc.tile_pool(name="stps", bufs=2, space="PSUM") as stps_pool,
            tc.tile_pool(name="acc", bufs=2, space="PSUM") as acc_pool,
        ):
            ident = const_pool.tile([128, 128], f32)
            masks.make_identity(nc, ident[:])
            const_pool.ident = ident
            ident_r = const_pool.tile([128, 128], f32r)
            nc.vector.tensor_copy(ident_r[:], ident[:])
            const_pool.ident_r = ident_r
            # warm the Exp activation table off the critical path
            warm = const_pool.tile([128, 1], f32)
            nc.vector.memset(warm[:], 0.0)
            warm2 = const_pool.tile([128, 1], f32)
            nc.scalar.activation(warm2[:], warm[:], EXP)
            # warm the PE clock (HAM) during the first DMA window with dummy
            # matmuls on a zeroed tile so stage-A transposes run at full rate
            wmm = const_pool.tile([128, 128], mybir.dt.bfloat16)
            nc.vector.memset(wmm[:], 0.0)
            wps = acc_pool.tile([128, 512], f32, tag="acc")
            for wi in range(10):
                nc.tensor.matmul(
                    wps[:, 0:128], wmm[:], wmm[:], start=True, stop=True
                )
            pools = (const_pool, qkv_pool, vr_pool, qt_pool, expt_pool, e_pool,
                     p_pool, cts_pool, ctx_pool, sc_pool, sps_pool, stps_pool,
                     acc_pool)
            for r in range(reps):
                stages = [_emit_stage_a(nc, pools, dram, b) for b in range(BPC)]
                for b in range(BPC):
                    _emit_blocks(nc, pools, dram, b, stages[b],
                                 last=(r == reps - 1 and b == BPC - 1))

    nc.compile()
    _cache[reps] = nc
    return nc


def run(q, k, v, reps=1, **spmd_kwargs):
    nc = _build(reps)
    q = np.ascontiguousarray(np.asarray(q, dtype=np.float32))
    k = np.ascontiguousarray(np.asarray(k, dtype=np.float32))
    v = np.ascontiguousarray(np.asarray(v, dtype=np.float32))
    in_maps = [
        {
            "q": q[c * BPC : (c + 1) * BPC],
            "k": k[c * BPC : (c + 1) * BPC],
            "v": v[c * BPC : (c + 1) * BPC],
        }
        for c in range(NCORES)
    ]
    out = run_bass_kernel_spmd(nc, in_maps, core_ids=list(range(NCORES)), **spmd_kwargs)
    ctx = np.concatenate([r["ctx"] for r in out.results], axis=0)
    probs = np.concatenate([r["probs"] for r in out.results], axis=0)
    return (ctx, probs), out


def kernel(q, k, v):
    (ctx, probs), _ = run(q, k, v)
    return (ctx, probs)


# revision 64
# speedup vs baseline: 1.0160x; 1.0007x over previous
"""Trainium2 Bass kernel: batched scaled-dot-product attention returning
(ctx, probs), data-parallel over 8 NeuronCores (2 batches/core).

Per-core dataflow (B=2 local batches, L=2048, D=128):
  - load q,k,v; PE-transpose q,k -> qT,kT [128d, L] in SBUF (float32r)
  - probs path, per 128-row q-tile: scores = qT_i^T @ kT (fp32r matmuls into
    PSUM), ScalarE exp(scale*x) with accum_out giving row sums, VectorE
    normalize by reciprocal, DMA out.
  - ctx path, per 512-col q-block: build expT [k, q] either by
    (V2) PE-transposing the exp tiles, evacuating PSUM->SBUF on ScalarE/
    VectorE alternately, or (V1) recomputing scoresT = kT_j^T @ qT on the PE
    and running a second exp pass; then ctxT += v_j^T . expT_j accumulated in
    one PSUM bank; PE-transpose back to [q, d], VectorE scale by the
    reciprocal, DMA out.
"""

import numpy as np

import concourse.bass as bass
import concourse.tile as tile
from concourse import bacc, masks, mybir
from concourse.bass_utils import run_bass_kernel_spmd

B, L, D = 16, 2048, 128
NCORES = 8
BPC = B // NCORES          # batches per core
NT = L // 128              # 128-row tiles per batch (16)
QB = 512                   # q-block width for the ctx path
NQB = L // QB              # q-blocks per batch (4)
SCALE = float(np.float32(1.0) / np.sqrt(np.float32(D)))

f32 = mybir.dt.float32
f32r = mybir.dt.float32r
EXP = mybir.ActivationFunctionType.Exp

# ctx-path expT source per k-tile j: j in TRANS_JS -> PE-transpose of the exp1
# output (evacuated on VectorE); else -> scoresT matmul + second exp on ScalarE.
# The 8/8 split balances ScalarE and VectorE; measured optimum.
TRANS_JS = frozenset(range(8, 16))

_cache = {}


def _emit_stage_a(nc, pools, dram, b):
    (const_pool, qkv_pool, vr_pool, qt_pool, expt_pool, e_pool, p_pool,
     cts_pool, ctx_pool, sc_pool, sps_pool, stps_pool, acc_pool) = pools
    q_d, k_d, v_d, probs_d, ctx_d = dram
    ident = const_pool.ident

    # ---- load q,k,v; first-needed chunks first so compute starts early ----
    qsb = qkv_pool.tile([128, NT, 128], f32, tag="qsb")
    ksb = qkv_pool.tile([128, NT, 128], f32, tag="ksb")
    vsb = qkv_pool.tile([128, NT, 128], f32, tag="vsb")
    q_ap = q_d[b].rearrange("(t p) d -> p t d", p=128)
    k_ap = k_d[b].rearrange("(t p) d -> p t d", p=128)
    v_ap = v_d[b].rearrange("(t p) d -> p t d", p=128)
    loads = [(k_ap, ksb, 0), (q_ap, qsb, 0), (k_ap, ksb, 1), (k_ap, ksb, 2),
             (k_ap, ksb, 3)]
    loads += [(q_ap, qsb, c) for c in range(1, 4)]
    loads += [(v_ap, vsb, c) for c in range(4)]
    for ap, dst, c in loads:
        nc.sync.dma_start(
            dst[:, c * 4 : (c + 1) * 4, :], ap[:, c * 4 : (c + 1) * 4, :]
        )

    # ---- PE-transpose q,k into [128 d, L] f32r ----
    qTr = qt_pool.tile([128, L], f32r, tag="qTr")
    kTr = qt_pool.tile([128, L], f32r, tag="kTr")
    tgroups = [(ksb, kTr, 0), (qsb, qTr, 0), (ksb, kTr, 1), (ksb, kTr, 2),
               (ksb, kTr, 3)]
    tgroups += [(qsb, qTr, g) for g in range(1, 4)]
    for sb_t, dst, g in tgroups:
        tp = acc_pool.tile([128, 512], f32, tag="acc")
        for u in range(4):
            t = g * 4 + u
            nc.tensor.transpose(
                tp[:, u * 128 : (u + 1) * 128], sb_t[:, t, :], ident[:]
            )
        nc.vector.tensor_copy(dst[:, g * 512 : (g + 1) * 512], tp[:])

    # cast v after the q/k transpose evacs: v loads land last, and an early
    # cast would head-of-line block the DVE queue behind the v DMA
    v_r = vr_pool.tile([128, NT, 128], f32r)
    nc.vector.tensor_copy(v_r[:], vsb[:])
    return qTr, kTr, v_r


def _ctx_out(nc, pools, ctx_d, b, qb, iis, ctp, rrows):
    """Scale the transposed ctx tiles for q-tiles `iis` (contiguous) and
    store them with one merged DMA (interleaved-row 3D access pattern)."""
    ctx_pool = pools[8]
    nt = len(iis)
    ct = ctx_pool.tile([128, nt, 128], f32, tag="ct")
    for x, ii in enumerate(iis):
        nc.vector.tensor_scalar_mul(
            ct[:, x, :], ctp[:, x * 128 : (x + 1) * 128], rrows[ii][:]
        )
    i0 = qb * 4 + iis[0]
    dst = ctx_d[b, i0 * 128 : (i0 + nt) * 128, :].rearrange(
        "(c p) d -> p c d", p=128
    )
    nc.sync.dma_start(dst, ct[:])


def _emit_blocks(nc, pools, dram, b, stage, last):
    (const_pool, qkv_pool, vr_pool, qt_pool, expt_pool, e_pool, p_pool,
     cts_pool, ctx_pool, sc_pool, sps_pool, stps_pool, acc_pool) = pools
    q_d, k_d, v_d, probs_d, ctx_d = dram
    qTr, kTr, v_r = stage
    ident = const_pool.ident
    trans_js = sorted(TRANS_JS)
    mm_js = sorted(set(range(NT)) - TRANS_JS)

    def sT_exp2(qb):
        """scoresT matmul + second exp for the matmul-route k-tiles; returns
        this block's expT tile and ctx accumulator halves."""
        expT = expt_pool.tile([128, NT * QB], f32r)
        if last and qb == NQB - 1:
            # split ctx accumulator: half A (q 256:512) closes early so its
            # output chain overlaps the remaining exp work
            cTa = acc_pool.tile([128, QB // 2], f32, tag="acc")
            cTb = acc_pool.tile([128, QB // 2], f32, tag="acc")
            cT = (cTa, cTb)
        else:
            cTf = acc_pool.tile([128, QB], f32, tag="acc")
            cT = (cTf,)
        for j in mm_js:
            sT = stps_pool.tile([128, QB], f32, tag="stp")
            nc.tensor.matmul(
                sT[:],
                kTr[:, j * 128 : (j + 1) * 128],
                qTr[:, qb * QB : (qb + 1) * QB],
                start=True,
                stop=True,
            )
            nc.scalar.activation(
                expT[:, j * QB : (j + 1) * QB], sT[:], EXP, scale=SCALE
            )
        return expT, cT

    def fold(cTt, expT, js, col0, colw, start, stop):
        for jx, j in enumerate(js):
            nc.tensor.matmul(
                cTt[:, 0:colw],
                v_r[:, j, :],
                expT[:, j * QB + col0 : j * QB + col0 + colw],
                start=(start and jx == 0),
                stop=(stop and jx == len(js) - 1),
            )

    def ctx_chain(cTt, expT, qb, col0, colw, iis, rrows):
        """trans-folds (closing the group) + evacuate + transpose + scale/DMA
        for q columns [col0, col0+colw) of the block."""
        fold(cTt, expT, trans_js, col0, colw, start=False, stop=True)
        cT_sb = cts_pool.tile([128, QB], f32)
        nc.vector.tensor_copy(cT_sb[:, 0:colw], cTt[:, 0:colw])
        ctp = stps_pool.tile([128, QB], f32, tag="stp")
        for x in range(colw // 128):
            nc.tensor.transpose(
                ctp[:, x * 128 : (x + 1) * 128],
                cT_sb[:, x * 128 : (x + 1) * 128],
                ident[:],
            )
        _ctx_out(nc, pools, ctx_d, b, qb, iis, ctp, rrows)

    state = sT_exp2(0)
    for qb in range(NQB):
        last_blk = last and qb == NQB - 1
        expT, cT = state
        expT_v = expT[:].rearrange("p (j q) -> p j q", q=QB)
        rrows = {}

        # ---- probs path: one 128-row q-tile at a time ----
        # scores are computed in two [128, 1024] half-tiles (2 PSUM banks
        # each, double-buffered) so the PE can fill half B / next tile's
        # half A while ScalarE is still reading the previous half.
        ii_order = [3, 2, 1, 0] if last_blk else [0, 1, 2, 3]
        for pos, ii in enumerate(ii_order):
            i = qb * 4 + ii
            ebuf = e_pool.tile([128, L], f32r)
            # the very first tile uses 512-wide exp chunks so the first
            # activation starts as soon as one k-transpose group is ready
            fine = b == 0 and qb == 0 and ii == 0
            zparts = []
            for h in range(2):
                s_ps = sps_pool.tile([128, L // 2], f32, tag="sps")
                for n in range(2):
                    nc.tensor.matmul(
                        s_ps[:, n * 512 : (n + 1) * 512],
                        qTr[:, i * 128 : (i + 1) * 128],
                        kTr[:, (2 * h + n) * 512 : (2 * h + n + 1) * 512],
                        start=True,
                        stop=True,
                    )
                    if fine:
                        zh = sc_pool.tile([128, 1], f32, tag="zrow")
                        nc.scalar.activation(
                            ebuf[:, (2 * h + n) * 512 : (2 * h + n + 1) * 512],
                            s_ps[:, n * 512 : (n + 1) * 512],
                            EXP,
                            scale=SCALE,
                            accum_out=zh[:],
                        )
                        zparts.append(zh)
                if not fine:
                    zh = sc_pool.tile([128, 1], f32, tag="zrow")
                    nc.scalar.activation(
                        ebuf[:, h * (L // 2) : (h + 1) * (L // 2)],
                        s_ps[:],
                        EXP,
                        scale=SCALE,
                        accum_out=zh[:],
                    )
                    zparts.append(zh)
            while len(zparts) > 1:
                zn = sc_pool.tile([128, 1], f32, tag="zsum")
                nc.vector.tensor_add(zn[:], zparts[0][:], zparts[1][:])
                zparts = zparts[2:] + [zn]
            rrow = sc_pool.tile([128, 1], f32, tag="rrow")
            nc.vector.reciprocal(rrow[:], zparts[0][:])
            rrows[ii] = rrow

            def norm_dma(nch):
                pt = p_pool.tile([128, L], f32, tag="pt")
                cw = L // nch
                for ch in range(nch):
                    nc.vector.tensor_scalar_mul(
                        pt[:, ch * cw : (ch + 1) * cw],
                        ebuf[:, ch * cw : (ch + 1) * cw],
                        rrow[:],
                    )
                    nc.sync.dma_start(
                        probs_d[b, i * 128 : (i + 1) * 128,
                                ch * cw : (ch + 1) * cw],
                        pt[:, ch * cw : (ch + 1) * cw],
                    )

            def trans_evac():
                for g in range(len(trans_js) // 4):
                    js = trans_js[g * 4 : g * 4 + 4]
                    tp = stps_pool.tile([128, 512], f32r, tag="stp")
                    for u, j in enumerate(js):
                        nc.tensor.transpose(
                            tp[:, u * 128 : (u + 1) * 128],
                            ebuf[:, j * 128 : (j + 1) * 128],
                            const_pool.ident_r[:],
                        )
                    dst = expT_v[:, js[0] : js[0] + 4, ii * 128 : (ii + 1) * 128]
                    src = tp[:].rearrange("p (u x) -> p u x", x=128)
                    nc.vector.tensor_copy(dst, src)

            if last_blk and pos == 3:
                # final tile: evacuate first (it gates the last ctx fold),
                # then chunked normalize+DMA, then the half-A output chain
                trans_evac()
                norm_dma(2)
                ctx_chain(cT[0], expT, qb, 256, 256, [2, 3], rrows)
            else:
                norm_dma(1)
                if pos == 1:
                    # fold the matmul-route k-tiles into the ctx accumulator
                    # now: their exp2 outputs are long done, no PE HOL wait
                    if last_blk:
                        fold(cT[0], expT, mm_js, 256, 256, start=True,
                             stop=False)
                        fold(cT[1], expT, mm_js, 0, 256, start=True,
                             stop=False)
                    else:
                        fold(cT[0], expT, mm_js, 0, QB, start=True, stop=False)
                trans_evac()

        # next block's scoresT/exp2 run before this block's trailing folds so
        # the PE/ACT never idle at the block boundary
        if qb + 1 < NQB:
            state = sT_exp2(qb + 1)

        if last_blk:
            ctx_chain(cT[1], expT, qb, 0, 256, [0, 1], rrows)
        else:
            ctx_chain(cT[0], expT, qb, 0, QB, [0, 1, 2, 3], rrows)


def _build(reps=1):
    if reps in _cache:
        return _cache[reps]

    nc = bacc.Bacc("TRN2", target_bir_lowering=False, debug=False)
    q_d = nc.dram_tensor("q", [BPC, L, D], f32, kind="ExternalInput").ap()
    k_d = nc.dram_tensor("k", [BPC, L, D], f32, kind="ExternalInput").ap()
    v_d = nc.dram_tensor("v", [BPC, L, D], f32, kind="ExternalInput").ap()
    probs_d = nc.dram_tensor("probs", [BPC, L, L], f32, kind="ExternalOutput").ap()
    ctx_d = nc.dram_tensor("ctx", [BPC, L, D], f32, kind="ExternalOutput").ap()
    dram = (q_d, k_d, v_d, probs_d, ctx_d)

    with tile.TileContext(nc) as tc:
        with (
            tc.tile_pool(name="const", bufs=1) as const_pool,
            tc.tile_pool(name="qkv", bufs=1) as qkv_pool,
            tc.tile_pool(name="vr", bufs=2) as vr_pool,
            tc.tile_pool(name="qt", bufs=2) as qt_pool,
            tc.tile_pool(name="expt", bufs=2) as expt_pool,
            tc.tile_pool(name="ebuf", bufs=3) as e_pool,
            tc.tile_pool(name="probs", bufs=4) as p_pool,
            tc.tile_pool(name="cts", bufs=2) as cts_pool,
            tc.tile_pool(name="ctxo", bufs=2) as ctx_pool,
            tc.tile_pool(name="scal", bufs=24) as sc_pool,
            tc.tile_pool(name="sps", bufs=2, space="PSUM") as sps_pool,
            tc.tile_pool(name="stps", bufs=2, space="PSUM") as stps_pool,
            tc.tile_pool(name="acc", bufs=2, space="PSUM") as acc_pool,
        ):
            ident = const_pool.tile([128, 128], f32)
            masks.make_identity(nc, ident[:])
            const_pool.ident = ident
            ident_r = const_pool.tile([128, 128], f32r)
            nc.vector.tensor_copy(ident_r[:], ident[:])
            const_pool.ident_r = ident_r
            # warm the Exp activation table off the critical path
            warm = const_pool.tile([128, 1], f32)
            nc.vector.memset(warm[:], 0.0)
            warm2 = const_pool.tile([128, 1], f32)
            nc.scalar.activation(warm2[:], warm[:], EXP)
            # warm the PE clock (HAM) during the first DMA window with dummy
            # matmuls on a zeroed tile so stage-A transposes run at full rate
            wmm = const_pool.tile([128, 128], mybir.dt.bfloat16)
            nc.vector.memset(wmm[:], 0.0)
            wps = acc_pool.tile([128, 512], f32, tag="acc")
            for wi in range(24):
                nc.tensor.matmul(
                    wps[:, 0:128], wmm[:], wmm[:], start=True, stop=True
                )
            pools = (const_pool, qkv_pool, vr_pool, qt_pool, expt_pool, e_pool,
                     p_pool, cts_pool, ctx_pool, sc_pool, sps_pool, stps_pool,
                     acc_pool)
            for r in range(reps):
                stages = [_emit_stage_a(nc, pools, dram, b) for b in range(BPC)]
                for b in range(BPC):
                    _emit_blocks(nc, pools, dram, b, stages[b],
                                 last=(r == reps - 1 and b == BPC - 1))

    nc.compile()
    _cache[reps] = nc
    return nc


def run(q, k, v, reps=1, **spmd_kwargs):
    nc = _build(reps)
    q = np.ascontiguousarray(np.asarray(q, dtype=np.float32))
    k = np.ascontiguousarray(np.asarray(k, dtype=np.float32))
    v = np.ascontiguousarray(np.asarray(v, dtype=np.float32))
    in_maps = [
        {
            "q": q[c * BPC : (c + 1) * BPC],
            "k": k[c * BPC : (c + 1) * BPC],
            "v": v[c * BPC : (c + 1) * BPC],
        }
        for c in range(NCORES)
    ]
    out = run_bass_kernel_spmd(nc, in_maps, core_ids=list(range(NCORES)), **spmd_kwargs)
    ctx = np.concatenate([r["ctx"] for r in out.results], axis=0)
    probs = np.concatenate([r["probs"] for r in out.results], axis=0)
    return (ctx, probs), out


def kernel(q, k, v):
    (ctx, probs), _ = run(q, k, v)
    return (ctx, probs)
